# revision 1
# baseline (speedup 1.0000x reference)
"""Trainium2 Bass kernel for a sparse-attention EncoderLayer.

Sharding: rows (L) are split into 8 contiguous shards of L/8; each edge is
owned by the core that owns its destination row (row_index is sorted, so each
core's edges are a contiguous range).  Each core computes Q/K/V for its row
shard, the K/V shards are AllGathered (bf16) so every core holds the full
K/V table in HBM, and per-edge K/V rows are fetched with dma_gather.  The
segment softmax is computed without the max-subtraction (scores here are
bounded by ~|q||k|/8 + |bias| < 10, so exp() cannot overflow in f32 and
alpha = exp(s - m)/sum exp(s - m) == exp(s)/sum exp(s)).  The alpha-weighted
scatter and the per-row softmax sums are evaluated as one-hot PE matmuls over
128-edge tiles, accumulated in PSUM per 128-row block.
"""

import math
import numpy as np
from contextlib import ExitStack

from ml_dtypes import bfloat16

import concourse.bass as bass
import concourse.mybir as mybir
import concourse.tile as tile
from concourse import bacc
from concourse.bass_utils import run_bass_kernel_spmd
from concourse.masks import make_identity

NCORES = 8
C, H, D, HID = 512, 8, 64, 1024
EPS = 1e-5
CHUNK_T = 16  # edge tiles (of 128 edges) per dma_gather chunk
F32 = mybir.dt.float32
BF16 = mybir.dt.bfloat16
I16 = mybir.dt.int16
AF = mybir.ActivationFunctionType
ALU = mybir.AluOpType
AX = mybir.AxisListType

_prog_cache = {}
TRACE = False          # set True (with the ntff hook registered) to profile
LAST_EXEC_NS = None    # exec time of the last run when TRACE was on
LAST_RESULTS = None    # full BassKernelResults of the last run


# --------------------------------------------------------------------------
# host-side preprocessing
# --------------------------------------------------------------------------

def _wrap_idx(idx):
    """[n] int -> [128, n//16] int16, wrapped (idx i at partition i%16,
    column i//16) and replicated across the 8 Q7 cores."""
    n = idx.shape[0]
    w = np.ascontiguousarray(idx.reshape(n // 16, 16).T).astype(np.int16)
    return np.tile(w, (8, 1))


def _preprocess_edges(L, row, col, att_bias):
    LSH = L // NCORES
    NBLK = LSH // 128
    bounds = np.searchsorted(row, np.arange(NCORES + 1) * LSH)

    per_core = []
    t_blk = 1
    for c in range(NCORES):
        e0, e1 = int(bounds[c]), int(bounds[c + 1])
        r = row[e0:e1] - c * LSH
        blk = r >> 7
        cnt = np.bincount(blk, minlength=NBLK)
        t_blk = max(t_blk, int(np.max((cnt + 127) // 128)) if len(cnt) else 1)
        per_core.append((e0, e1, r, blk, cnt))

    T_BLK = t_blk
    NT = NBLK * T_BLK
    NCH = (NT + CHUNK_T - 1) // CHUNK_T
    NTP = NCH * CHUNK_T  # tiles padded to whole chunks (extra tiles unused)

    cores = []
    for c in range(NCORES):
        e0, e1, r, blk, cnt = per_core[c]
        ne = e1 - e0
        starts = np.zeros(NBLK, dtype=np.int64)
        np.cumsum(cnt[:-1], out=starts[1:])
        idx_in_blk = np.arange(ne, dtype=np.int64) - starts[blk]
        dst = blk * (T_BLK * 128) + idx_in_blk

        npad = NTP * 128
        colP = np.zeros(npad, dtype=np.int64)
        qlocP = np.zeros(npad, dtype=np.int64)
        rlocP = np.zeros(npad, dtype=np.float32)
        biasP = np.full((npad, H), -30000.0, dtype=np.float32)
        colP[dst] = col[e0:e1]
        qlocP[dst] = r
        rlocP[dst] = (r & 127).astype(np.float32)
        biasP[dst] = att_bias[e0:e1]

        colw = _wrap_idx(colP).reshape(128, NCH, CHUNK_T * 8).transpose(1, 0, 2)
        qlocw = _wrap_idx(qlocP).reshape(128, NCH, CHUNK_T * 8).transpose(1, 0, 2)
        colw = colw.reshape(NCH * 128, CHUNK_T * 8)
        qlocw = qlocw.reshape(NCH * 128, CHUNK_T * 8)
        # [NT, 128, H] / [NT, 128] partition-major per tile
        biasT = biasP.reshape(NTP, 128, H)[:NT]
        rlocT = rlocP.reshape(NTP, 128)[:NT]
        cores.append(dict(
            colw=np.ascontiguousarray(colw),
            qlocw=np.ascontiguousarray(qlocw),
            biasP=np.ascontiguousarray(biasT),
            rlocP=np.ascontiguousarray(rlocT),
        ))
    return T_BLK, NT, NCH, cores


def _prep_weights(inp):
    scale = 1.0 / math.sqrt(D)

    def mat(w, kchunks):
        w = np.asarray(w, np.float32)
        k, n = w.shape
        assert k == kchunks * 128
        return np.ascontiguousarray(
            w.reshape(kchunks, 128, n).transpose(1, 0, 2)).astype(bfloat16)

    def rowv(b):
        return np.asarray(b, np.float32)[None, :].astype(bfloat16)

    return dict(
        wq=mat(np.asarray(inp["Wq"], np.float32) * scale, 4),
        wk=mat(inp["Wk"], 4),
        wv=mat(inp["Wv"], 4),
        wo=mat(inp["Wo"], 4),
        w1=mat(inp["W1"], 4),
        w2=mat(inp["W2"], 8),
        bq=rowv(np.asarray(inp["bq"], np.float32) * scale),
        bk=rowv(inp["bk"]), bv=rowv(inp["bv"]), bo=rowv(inp["bo"]),
        b1=rowv(inp["b1"]), b2=rowv(inp["b2"]),
        ln1g=np.asarray(inp["ln1_g"], np.float32),
        ln1b=np.asarray(inp["ln1_b"], np.float32),
        ln2g=np.asarray(inp["ln2_g"], np.float32),
        ln2b=np.asarray(inp["ln2_b"], np.float32),
    )


# --------------------------------------------------------------------------
# walrus workaround: this walrus build rejects Drain instructions carrying
# more than one sem wait ("Too many sync wait commands") -- split the extra
# waits onto NOPs inserted just before, on the same engine.
# --------------------------------------------------------------------------

def _split_multi_waits(nc):
    nid = [0]
    for fn in nc.m.functions:
        for blk in fn.blocks:
            insts = blk.instructions
            i = 0
            while i < len(insts):
                inst = insts[i]
                si = inst.sync_info
                if (isinstance(inst, mybir.InstDrain)
                        and si is not None and si.on_wait and len(si.on_wait) > 1):
                    waits = list(si.on_wait)
                    nops = []
                    for w in waits[:-1]:
                        nid[0] += 1
                        nops.append(mybir.InstNoOp(
                            name=f"I-waitfix-{nid[0]}",
                            engine=inst.engine, ins=[], outs=[],
                            sync_info=mybir.SyncInfo(on_wait=[w], on_update=[]),
                        ))
                    inst.sync_info = mybir.SyncInfo(
                        on_wait=[waits[-1]], on_update=list(si.on_update))
                    insts[i:i] = nops
                    i += len(nops)
                i += 1


# --------------------------------------------------------------------------
# device program
# --------------------------------------------------------------------------

def _bc(ap, n):
    """append a broadcast (step-0) innermost dim of size n to an AP"""
    return bass.AP(tensor=ap.tensor, offset=ap.offset, ap=[*ap.ap, [0, n]])


def _phd(ap):
    return ap.rearrange("p (h d) -> p h d", h=H)


def _build_program(L, T_BLK, NT, NCH):
    LSH = L // NCORES
    NBLK = LSH // 128
    nc = bacc.Bacc(num_devices=NCORES)

    x_c = nc.declare_dram_parameter("x_c", [LSH, C], F32, isOutput=False)
    wq = nc.declare_dram_parameter("wq", [128, 4, C], BF16, isOutput=False)
    wk = nc.declare_dram_parameter("wk", [128, 4, C], BF16, isOutput=False)
    wv = nc.declare_dram_parameter("wv", [128, 4, C], BF16, isOutput=False)
    wo = nc.declare_dram_parameter("wo", [128, 4, C], BF16, isOutput=False)
    w1 = nc.declare_dram_parameter("w1", [128, 4, HID], BF16, isOutput=False)
    w2 = nc.declare_dram_parameter("w2", [128, 8, C], BF16, isOutput=False)
    bqp = nc.declare_dram_parameter("bq", [1, C], BF16, isOutput=False)
    bkp = nc.declare_dram_parameter("bk", [1, C], BF16, isOutput=False)
    bvp = nc.declare_dram_parameter("bv", [1, C], BF16, isOutput=False)
    bop = nc.declare_dram_parameter("bo", [1, C], BF16, isOutput=False)
    b1p = nc.declare_dram_parameter("b1", [1, HID], BF16, isOutput=False)
    b2p = nc.declare_dram_parameter("b2", [1, C], BF16, isOutput=False)
    ln1g = nc.declare_dram_parameter("ln1g", [C], F32, isOutput=False)
    ln1b = nc.declare_dram_parameter("ln1b", [C], F32, isOutput=False)
    ln2g = nc.declare_dram_parameter("ln2g", [C], F32, isOutput=False)
    ln2b = nc.declare_dram_parameter("ln2b", [C], F32, isOutput=False)
    colw = nc.declare_dram_parameter("colw", [NCH * 128, CHUNK_T * 8], I16, isOutput=False)
    qlocw = nc.declare_dram_parameter("qlocw", [NCH * 128, CHUNK_T * 8], I16, isOutput=False)
    biasP = nc.declare_dram_parameter("biasP", [NT, 128, H], F32, isOutput=False)
    rlocP = nc.declare_dram_parameter("rlocP", [NT, 128], F32, isOutput=False)
    y_out = nc.declare_dram_parameter("y", [LSH, C], F32, isOutput=True)

    with ExitStack() as ctx:
        tc = ctx.enter_context(tile.TileContext(nc))

        dram = ctx.enter_context(tc.tile_pool(name="dram", bufs=1, space="DRAM"))
        q_tab = dram.tile([LSH, C], BF16)
        kv_sh = dram.tile([LSH, 2 * C], BF16)
        kv_full = dram.tile([NCORES * LSH, 2 * C], BF16)
        x1_d = dram.tile([LSH, C], F32)

        # ---------------- constants + weights ----------------
        consts = ctx.enter_context(tc.tile_pool(name="consts", bufs=1))
        ident = consts.tile([128, 128], BF16, tag="ident")
        make_identity(nc, ident[:])
        iota_row = consts.tile([128, 128], BF16, tag="iota")
        nc.gpsimd.iota(iota_row[:], pattern=[[1, 128]], base=0,
                       channel_multiplier=0,
                       allow_small_or_imprecise_dtypes=True)
        ones_k1 = consts.tile([1, 128], BF16, tag="ones")
        nc.vector.memset(ones_k1[:], 1.0)
        eps_t = consts.tile([128, 1], F32, tag="eps")
        nc.vector.memset(eps_t[:], EPS)

        def bcast_load(param, tag):
            t = consts.tile([128, C], F32, tag=tag)
            ap = param[:]
            src = bass.AP(tensor=ap.tensor, offset=ap.offset,
                          ap=[[0, 128], [1, C]])
            nc.sync.dma_start(out=t[:], in_=src)
            return t

        g1_bc, b1_bc = bcast_load(ln1g, "g1"), bcast_load(ln1b, "b1")
        g2_bc, b2_bc = bcast_load(ln2g, "g2"), bcast_load(ln2b, "b2")

        wts = ctx.enter_context(tc.tile_pool(name="wts", bufs=1))

        def wload(p, shape, tag):
            t = wts.tile(shape, BF16, tag=tag)
            nc.sync.dma_start(out=t[:], in_=p[:])
            return t

        wq_sb = wload(wq, [128, 4, C], "wq"); wk_sb = wload(wk, [128, 4, C], "wk")
        wv_sb = wload(wv, [128, 4, C], "wv"); wo_sb = wload(wo, [128, 4, C], "wo")
        w1_sb = wload(w1, [128, 4, HID], "w1"); w2_sb = wload(w2, [128, 8, C], "w2")
        bq_sb = wload(bqp, [1, C], "bq"); bk_sb = wload(bkp, [1, C], "bk")
        bv_sb = wload(bvp, [1, C], "bv"); bo_sb = wload(bop, [1, C], "bo")
        b1_sb = wload(b1p, [1, HID], "bb1"); b2_sb = wload(b2p, [1, C], "bb2")

        # ---------------- LN helper ----------------
        def layernorm(pool, lnpool, xb, g_bc, bb_bc):
            """returns bf16 [128, C] normalized tile"""
            stats = lnpool.tile([128, 6], F32, tag="stats")
            nc.vector.bn_stats(stats[:], xb[:])
            mv = lnpool.tile([128, 2], F32, tag="mv")
            nc.vector.bn_aggr(mv[:], stats[:])
            xc = pool.tile([128, C], F32, tag="ln_xc")
            nc.vector.tensor_scalar(xc[:], xb[:], mv[:, 0:1], None, op0=ALU.subtract)
            sd = lnpool.tile([128, 1], F32, tag="sd")
            nc.scalar.activation(sd[:], mv[:, 1:2], AF.Sqrt, bias=eps_t[:])
            rstd = lnpool.tile([128, 1], F32, tag="rstd")
            nc.vector.reciprocal(rstd[:], sd[:])
            z0 = pool.tile([128, C], F32, tag="ln_z0")
            nc.vector.tensor_scalar(z0[:], xc[:], rstd[:], None, op0=ALU.mult)
            z1 = pool.tile([128, C], F32, tag="ln_z1")
            nc.vector.tensor_mul(z1[:], z0[:], g_bc[:])
            zb = pool.tile([128, C], BF16, tag="ln_out")
            nc.vector.tensor_add(zb[:], z1[:], bb_bc[:])
            return zb

        # ---------------- phase B+C: LN1, zT, QKV ----------------
        with ExitStack() as pctx:
            zT_pool = pctx.enter_context(tc.tile_pool(name="zT", bufs=1))
            zT = zT_pool.tile([128, 4, LSH], BF16)
            xp = pctx.enter_context(tc.tile_pool(name="xp", bufs=3))
            lnp = pctx.enter_context(tc.tile_pool(name="lnp", bufs=4))
            trp = pctx.enter_context(tc.tile_pool(name="trp", bufs=2, space="PSUM"))
            qkvp = pctx.enter_context(tc.tile_pool(name="qkvp", bufs=2, space="PSUM"))
            obp = pctx.enter_context(tc.tile_pool(name="obp", bufs=3))

            for ib in range(NBLK):
                sl = slice(ib * 128, (ib + 1) * 128)
                xb = xp.tile([128, C], F32, tag="xin")
                nc.sync.dma_start(out=xb[:], in_=x_c[sl, :])
                zb = layernorm(xp, lnp, xb, g1_bc, b1_bc)
                for g in range(4):
                    pt = trp.tile([128, 128], BF16)
                    nc.tensor.transpose(pt[:], zb[:, g * 128:(g + 1) * 128], ident[:])
                    nc.scalar.copy(zT[:, g, sl], pt[:])
                for w_sb, bias_sb, dst in (
                    (wq_sb, bq_sb, None),
                    (wk_sb, bk_sb, 0),
                    (wv_sb, bv_sb, 1),
                ):
                    ps = qkvp.tile([128, C], F32)
                    for g in range(4):
                        nc.tensor.matmul(ps[:], lhsT=zT[:, g, sl], rhs=w_sb[:, g, :],
                                         start=(g == 0), stop=False)
                    nc.tensor.matmul(ps[:], lhsT=ones_k1[:], rhs=bias_sb[:],
                                     start=False, stop=True)
                    ob = obp.tile([128, C], BF16)
                    nc.scalar.copy(ob[:], ps[:])
                    if dst is None:
                        nc.sync.dma_start(out=q_tab[sl, :], in_=ob[:])
                    else:
                        nc.sync.dma_start(out=kv_sh[sl, dst * C:(dst + 1) * C], in_=ob[:])

        # ---------------- phase D: allgather K/V ----------------
        nc.gpsimd.collective_compute(
            "AllGather", ALU.bypass,
            replica_groups=[list(range(NCORES))],
            ins=[kv_sh[:]], outs=[kv_full[:]],
        )

        # ---------------- phase E: edges ----------------
        with ExitStack() as pctx:
            kvp = pctx.enter_context(tc.tile_pool(name="kvp", bufs=2))
            qgp = pctx.enter_context(tc.tile_pool(name="qgp", bufs=2))
            idxp = pctx.enter_context(tc.tile_pool(name="idxp", bufs=3))
            bp = pctx.enter_context(tc.tile_pool(name="bp", bufs=2))
            rlp = pctx.enter_context(tc.tile_pool(name="rlp", bufs=2))
            work = pctx.enter_context(tc.tile_pool(name="work", bufs=4))
            pop_ = pctx.enter_context(tc.tile_pool(name="pout", bufs=2, space="PSUM"))
            psp = pctx.enter_context(tc.tile_pool(name="pssum", bufs=1, space="PSUM"))
            trp2 = pctx.enter_context(tc.tile_pool(name="trp2", bufs=2, space="PSUM"))
            opp = pctx.enter_context(tc.tile_pool(name="opsum", bufs=1, space="PSUM"))
            finp = pctx.enter_context(tc.tile_pool(name="finp", bufs=2))

            kvb = qgb = bia = rlc = None
            pout = pssum = None
            for t in range(NT):
                ch, slot = divmod(t, CHUNK_T)
                if slot == 0:
                    tiles_c = min(CHUNK_T, NT - ch * CHUNK_T)
                    n_idx = tiles_c * 128
                    cidx = idxp.tile([128, CHUNK_T * 8], I16, tag="cidx")
                    nc.sync.dma_start(out=cidx[:], in_=colw[ch * 128:(ch + 1) * 128, :])
                    qidx = idxp.tile([128, CHUNK_T * 8], I16, tag="qidx")
                    nc.sync.dma_start(out=qidx[:], in_=qlocw[ch * 128:(ch + 1) * 128, :])
                    kvb = kvp.tile([128, CHUNK_T, 2 * C], BF16)
                    nc.gpsimd.dma_gather(
                        out_ap=kvb[:, :tiles_c, :], in_ap=kv_full[:],
                        idxs_ap=cidx[:, :n_idx // 16],
                        num_idxs=n_idx, num_idxs_reg=n_idx, elem_size=2 * C,
                        single_packet=False)
                    qgb = qgp.tile([128, CHUNK_T, C], BF16)
                    nc.gpsimd.dma_gather(
                        out_ap=qgb[:, :tiles_c, :], in_ap=q_tab[:],
                        idxs_ap=qidx[:, :n_idx // 16],
                        num_idxs=n_idx, num_idxs_reg=n_idx, elem_size=C,
                        single_packet=False)
                    bia = bp.tile([128, CHUNK_T, H], F32)
                    nc.sync.dma_start(
                        out=bia[:, :tiles_c, :],
                        in_=biasP[ch * CHUNK_T:ch * CHUNK_T + tiles_c, :, :]
                        .rearrange("t p h -> p t h"))
                    rlc = rlp.tile([128, CHUNK_T], F32)
                    nc.sync.dma_start(
                        out=rlc[:, :tiles_c],
                        in_=rlocP[ch * CHUNK_T:ch * CHUNK_T + tiles_c, :]
                        .rearrange("t p -> p t"))

                rb, tb = divmod(t, T_BLK)
                if tb == 0:
                    pout = pop_.tile([128, C], F32)
                    pssum = psp.tile([128, H], F32)

                kg = kvb[:, slot, 0:C]
                vg = kvb[:, slot, C:2 * C]
                qg = qgb[:, slot, :]
                prod = work.tile([128, C], BF16, tag="prod")
                nc.vector.tensor_mul(prod[:], kg, qg)
                sc = work.tile([128, H], F32, tag="sc")
                nc.vector.tensor_reduce(sc[:], _phd(prod[:]), axis=AX.X, op=ALU.add)
                sc2 = work.tile([128, H], F32, tag="sc2")
                nc.vector.tensor_add(sc2[:], sc[:], bia[:, slot, :])
                p_t = work.tile([128, H], BF16, tag="p")
                nc.scalar.activation(p_t[:], sc2[:], AF.Exp)
                oh = work.tile([128, 128], BF16, tag="oh")
                nc.vector.tensor_scalar(oh[:], iota_row[:], rlc[:, slot:slot + 1],
                                        None, op0=ALU.is_equal)
                wt = work.tile([128, C], BF16, tag="wt")
                nc.vector.tensor_tensor(_phd(wt[:]), _phd(vg), _bc(p_t[:], D),
                                        op=ALU.mult)
                nc.tensor.matmul(pout[:], lhsT=oh[:], rhs=wt[:],
                                 start=(tb == 0), stop=(tb == T_BLK - 1))
                nc.tensor.matmul(pssum[:], lhsT=oh[:], rhs=p_t[:],
                                 start=(tb == 0), stop=(tb == T_BLK - 1))

                if tb == T_BLK - 1:
                    sl = slice(rb * 128, (rb + 1) * 128)
                    sm = finp.tile([128, H], F32, tag="sm")
                    nc.vector.tensor_scalar(sm[:], pssum[:], 1e-30, None, op0=ALU.max)
                    rec = finp.tile([128, H], F32, tag="rec")
                    nc.vector.reciprocal(rec[:], sm[:])
                    att = finp.tile([128, C], BF16, tag="att")
                    nc.vector.tensor_tensor(_phd(att[:]), _phd(pout[:]),
                                            _bc(rec[:], D), op=ALU.mult)
                    attT = finp.tile([128, 4, 128], BF16, tag="attT")
                    for g in range(4):
                        pt = trp2.tile([128, 128], BF16)
                        nc.tensor.transpose(pt[:], att[:, g * 128:(g + 1) * 128], ident[:])
                        nc.scalar.copy(attT[:, g, :], pt[:])
                    po = opp.tile([128, C], F32)
                    for g in range(4):
                        nc.tensor.matmul(po[:], lhsT=attT[:, g, :], rhs=wo_sb[:, g, :],
                                         start=(g == 0), stop=False)
                    nc.tensor.matmul(po[:], lhsT=ones_k1[:], rhs=bo_sb[:],
                                     start=False, stop=True)
                    xb2 = finp.tile([128, C], F32, tag="xb2")
                    nc.sync.dma_start(out=xb2[:], in_=x_c[sl, :])
                    x1t = finp.tile([128, C], F32, tag="x1t")
                    nc.vector.tensor_add(x1t[:], po[:], xb2[:])
                    nc.sync.dma_start(out=x1_d[sl, :], in_=x1t[:])

        # ---------------- phase F: LN2 + MLP ----------------
        with ExitStack() as pctx:
            xp = pctx.enter_context(tc.tile_pool(name="xp2", bufs=3))
            lnp = pctx.enter_context(tc.tile_pool(name="lnp2", bufs=4))
            trp3 = pctx.enter_context(tc.tile_pool(name="trp3", bufs=2, space="PSUM"))
            hp = pctx.enter_context(tc.tile_pool(name="hpsum", bufs=1, space="PSUM"))
            yp = pctx.enter_context(tc.tile_pool(name="ypsum", bufs=1, space="PSUM"))
            sbp = pctx.enter_context(tc.tile_pool(name="sbp", bufs=3))

            for ib in range(NBLK):
                sl = slice(ib * 128, (ib + 1) * 128)
                x1t = xp.tile([128, C], F32, tag="x1in")
                nc.sync.dma_start(out=x1t[:], in_=x1_d[sl, :])
                z2 = layernorm(xp, lnp, x1t, g2_bc, b2_bc)
                z2T = sbp.tile([128, 4, 128], BF16, tag="z2T")
                for g in range(4):
                    pt = trp3.tile([128, 128], BF16)
                    nc.tensor.transpose(pt[:], z2[:, g * 128:(g + 1) * 128], ident[:])
                    nc.scalar.copy(z2T[:, g, :], pt[:])
                ph = hp.tile([128, 8, 128], F32)
                for chc in range(8):
                    csl = slice(chc * 128, (chc + 1) * 128)
                    for g in range(4):
                        nc.tensor.matmul(ph[:, chc, :], lhsT=w1_sb[:, g, csl],
                                         rhs=z2T[:, g, :], start=(g == 0), stop=False)
                    nc.tensor.matmul(ph[:, chc, :], lhsT=b1_sb[:, csl],
                                     rhs=ones_k1[:], start=False, stop=True)
                hs = sbp.tile([128, 8, 128], BF16, tag="hs")
                nc.scalar.activation(hs[:], ph[:], AF.Silu)
                py = yp.tile([128, C], F32)
                for chc in range(8):
                    nc.tensor.matmul(py[:], lhsT=hs[:, chc, :], rhs=w2_sb[:, chc, :],
                                     start=(chc == 0), stop=False)
                nc.tensor.matmul(py[:], lhsT=ones_k1[:], rhs=b2_sb[:],
                                 start=False, stop=True)
                yt = sbp.tile([128, C], F32, tag="yt")
                nc.vector.tensor_add(yt[:], py[:], x1t[:])
                nc.sync.dma_start(out=y_out[sl, :], in_=yt[:])

    nc.finalize()
    _split_multi_waits(nc)
    return nc


# --------------------------------------------------------------------------
# entry point
# --------------------------------------------------------------------------

def kernel(**inputs) -> np.ndarray:
    x = np.asarray(inputs["x"], np.float32)
    row = np.asarray(inputs["row_index"]).astype(np.int64)
    col = np.asarray(inputs["col_index"]).astype(np.int64)
    att_bias = np.asarray(inputs["att_bias"], np.float32)
    L = x.shape[0]
    LSH = L // NCORES

    T_BLK, NT, NCH, cores = _preprocess_edges(L, row, col, att_bias)
    w = _prep_weights(inputs)

    key = (L, T_BLK, NT, NCH)
    if key not in _prog_cache:
        _prog_cache[key] = _build_program(L, T_BLK, NT, NCH)
    nc = _prog_cache[key]

    in_maps = []
    for c in range(NCORES):
        m = dict(w)
        m["x_c"] = np.ascontiguousarray(x[c * LSH:(c + 1) * LSH])
        m.update(cores[c])
        in_maps.append(m)

    global LAST_EXEC_NS, LAST_RESULTS
    res = run_bass_kernel_spmd(nc, in_maps, list(range(NCORES)), trace=TRACE)
    LAST_RESULTS = res
    LAST_EXEC_NS = res.exec_time_ns
    return np.concatenate([res.results[c]["y"] for c in range(NCORES)], axis=0)



# revision 5
# speedup vs baseline: 1.3872x; 1.3872x over previous
"""Trainium2 Bass kernel for a sparse-attention EncoderLayer.

Sharding: rows (L) split into 8 contiguous shards of L/8; each edge is owned
by the core that owns its destination row (row_index is sorted, so each
core's edges are a contiguous range).  Each core computes Q/K/V for its row
shard; K/V shards are AllGathered (bf16, in 4 overlapping chunks) so every
core holds the full K/V table in HBM; per-edge K/V and Q rows are fetched
with dma_gather.  Segment softmax runs without max-subtraction (scores are
bounded, exp cannot overflow in f32).  Per-edge one-hot row selectors are
precomputed on the host and DMA'd in; the alpha-weighted scatter and softmax
sums are one-hot PE matmuls accumulated in PSUM per 128-row block.  The
LN2+MLP tail is fused into the edge phase per finished block.

DVE diet relative to the first version: the one-hot build, the p-broadcast
expansion and the x1 HBM roundtrip are gone; per-edge math is batched per
gather chunk (one DVE op per chunk instead of per 128-edge tile).
"""

import math
import numpy as np
from contextlib import ExitStack

from ml_dtypes import bfloat16

import concourse.bass as bass
import concourse.mybir as mybir
import concourse.tile as tile
from concourse import bacc
from concourse.bass_utils import run_bass_kernel_spmd
from concourse.masks import make_identity

NCORES = 8
C, H, D, HID = 512, 8, 64, 1024
EPS = 1e-5
CHUNK_T = 8   # edge tiles (of 128 edges) per dma_gather chunk
NAG = 4       # allgather chunks
F32 = mybir.dt.float32
BF16 = mybir.dt.bfloat16
I16 = mybir.dt.int16
AF = mybir.ActivationFunctionType
ALU = mybir.AluOpType
AX = mybir.AxisListType

_prog_cache = {}
TRACE = False
LAST_EXEC_NS = None
LAST_RESULTS = None


# --------------------------------------------------------------------------
# host-side preprocessing
# --------------------------------------------------------------------------

def _nag(NBLK):
    return NAG if NBLK % NAG == 0 else 1


def _wrap_idx(idx):
    """[n] int -> [128, n//16] int16, wrapped (idx i at partition i%16,
    column i//16) and replicated across the 8 Q7 cores."""
    n = idx.shape[0]
    w = np.ascontiguousarray(idx.reshape(n // 16, 16).T).astype(np.int16)
    return np.tile(w, (8, 1))


def _preprocess_edges(L, row, col, att_bias):
    LSH = L // NCORES
    NBLK = LSH // 128
    bounds = np.searchsorted(row, np.arange(NCORES + 1) * LSH)

    per_core = []
    t_blk = 1
    for c in range(NCORES):
        e0, e1 = int(bounds[c]), int(bounds[c + 1])
        r = row[e0:e1] - c * LSH
        blk = r >> 7
        cnt = np.bincount(blk, minlength=NBLK)
        t_blk = max(t_blk, int(np.max((cnt + 127) // 128)) if len(cnt) else 1)
        per_core.append((e0, e1, r, blk, cnt))

    T_BLK = t_blk
    NT = NBLK * T_BLK
    NCH = (NT + CHUNK_T - 1) // CHUNK_T
    NTP = NCH * CHUNK_T
    LSH4 = LSH // _nag(NBLK)

    cores = []
    for c in range(NCORES):
        e0, e1, r, blk, cnt = per_core[c]
        ne = e1 - e0
        starts = np.zeros(NBLK, dtype=np.int64)
        np.cumsum(cnt[:-1], out=starts[1:])
        idx_in_blk = np.arange(ne, dtype=np.int64) - starts[blk]
        dst = blk * (T_BLK * 128) + idx_in_blk

        npad = NTP * 128
        # col: global node id -> kv_full row (allgather chunk-major layout)
        gcol = col[e0:e1]
        oc, loc = gcol // LSH, gcol % LSH
        kvrow = (loc // LSH4) * (NCORES * LSH4) + oc * LSH4 + (loc % LSH4)
        colP = np.zeros(npad, dtype=np.int64)
        qlocP = np.zeros(npad, dtype=np.int64)
        rlocP = np.zeros(npad, dtype=np.int64)
        biasP = np.full((npad, H), -30000.0, dtype=np.float32)
        colP[dst] = kvrow
        qlocP[dst] = r
        rlocP[dst] = r & 127
        biasP[dst] = att_bias[e0:e1]

        colw = _wrap_idx(colP).reshape(128, NCH, CHUNK_T * 8).transpose(1, 0, 2)
        qlocw = _wrap_idx(qlocP).reshape(128, NCH, CHUNK_T * 8).transpose(1, 0, 2)
        colw = colw.reshape(NCH * 128, CHUNK_T * 8)
        qlocw = qlocw.reshape(NCH * 128, CHUNK_T * 8)
        # one-hot row selector per edge: [NT, 128, 128] bf16 via u16 bit trick
        ohu = np.zeros((NTP * 128, 128), dtype=np.uint16)
        ohu[dst, rlocP[dst]] = 0x3F80  # bf16 1.0
        oh = ohu.view(bfloat16).reshape(NTP, 128, 128)[:NT]
        # [NT, 128, H] bias, bf16
        biasT = biasP.reshape(NTP, 128, H)[:NT].astype(bfloat16)
        cores.append(dict(
            colw=np.ascontiguousarray(colw),
            qlocw=np.ascontiguousarray(qlocw),
            biasP=np.ascontiguousarray(biasT),
            ohP=np.ascontiguousarray(oh),
        ))
    return T_BLK, NT, NCH, cores


def _prep_weights(inp):
    scale = 1.0 / math.sqrt(D)

    def mat(w, kchunks):
        w = np.asarray(w, np.float32)
        k, n = w.shape
        assert k == kchunks * 128
        return np.ascontiguousarray(
            w.reshape(kchunks, 128, n).transpose(1, 0, 2)).astype(bfloat16)

    def rowv(b):
        return np.asarray(b, np.float32)[None, :].astype(bfloat16)

    return dict(
        wq=mat(np.asarray(inp["Wq"], np.float32) * scale, 4),
        wk=mat(inp["Wk"], 4),
        wv=mat(inp["Wv"], 4),
        wo=mat(inp["Wo"], 4),
        w1=mat(inp["W1"], 4),
        w2=mat(inp["W2"], 8),
        bq=rowv(np.asarray(inp["bq"], np.float32) * scale),
        bk=rowv(inp["bk"]), bv=rowv(inp["bv"]), bo=rowv(inp["bo"]),
        b1=rowv(inp["b1"]), b2=rowv(inp["b2"]),
        ln1g=np.asarray(inp["ln1_g"], np.float32)[None, :].astype(bfloat16),
        ln1b=np.asarray(inp["ln1_b"], np.float32)[None, :].astype(bfloat16),
        ln2g=np.asarray(inp["ln2_g"], np.float32)[None, :].astype(bfloat16),
        ln2b=np.asarray(inp["ln2_b"], np.float32)[None, :].astype(bfloat16),
    )


# --------------------------------------------------------------------------
# walrus workaround: split Drain instructions carrying >1 sem wait
# --------------------------------------------------------------------------

def _split_multi_waits(nc):
    nid = [0]
    for fn in nc.m.functions:
        for blk in fn.blocks:
            insts = blk.instructions
            i = 0
            while i < len(insts):
                inst = insts[i]
                si = inst.sync_info
                if (isinstance(inst, mybir.InstDrain)
                        and si is not None and si.on_wait and len(si.on_wait) > 1):
                    waits = list(si.on_wait)
                    nops = []
                    for w in waits[:-1]:
                        nid[0] += 1
                        nops.append(mybir.InstNoOp(
                            name=f"I-waitfix-{nid[0]}",
                            engine=inst.engine, ins=[], outs=[],
                            sync_info=mybir.SyncInfo(on_wait=[w], on_update=[]),
                        ))
                    inst.sync_info = mybir.SyncInfo(
                        on_wait=[waits[-1]], on_update=list(si.on_update))
                    insts[i:i] = nops
                    i += len(nops)
                i += 1


# --------------------------------------------------------------------------
# device program
# --------------------------------------------------------------------------

def _build_program(L, T_BLK, NT, NCH):
    LSH = L // NCORES
    NBLK = LSH // 128
    nag = _nag(NBLK)
    LSH4 = LSH // nag
    BPA = NBLK // nag  # blocks per allgather chunk
    nc = bacc.Bacc(num_devices=NCORES)

    x_c = nc.declare_dram_parameter("x_c", [LSH, C], F32, isOutput=False)
    wq = nc.declare_dram_parameter("wq", [128, 4, C], BF16, isOutput=False)
    wk = nc.declare_dram_parameter("wk", [128, 4, C], BF16, isOutput=False)
    wv = nc.declare_dram_parameter("wv", [128, 4, C], BF16, isOutput=False)
    wo = nc.declare_dram_parameter("wo", [128, 4, C], BF16, isOutput=False)
    w1 = nc.declare_dram_parameter("w1", [128, 4, HID], BF16, isOutput=False)
    w2 = nc.declare_dram_parameter("w2", [128, 8, C], BF16, isOutput=False)
    bqp = nc.declare_dram_parameter("bq", [1, C], BF16, isOutput=False)
    bkp = nc.declare_dram_parameter("bk", [1, C], BF16, isOutput=False)
    bvp = nc.declare_dram_parameter("bv", [1, C], BF16, isOutput=False)
    bop = nc.declare_dram_parameter("bo", [1, C], BF16, isOutput=False)
    b1p = nc.declare_dram_parameter("b1", [1, HID], BF16, isOutput=False)
    b2p = nc.declare_dram_parameter("b2", [1, C], BF16, isOutput=False)
    ln1g = nc.declare_dram_parameter("ln1g", [1, C], BF16, isOutput=False)
    ln1b = nc.declare_dram_parameter("ln1b", [1, C], BF16, isOutput=False)
    ln2g = nc.declare_dram_parameter("ln2g", [1, C], BF16, isOutput=False)
    ln2b = nc.declare_dram_parameter("ln2b", [1, C], BF16, isOutput=False)
    colw = nc.declare_dram_parameter("colw", [NCH * 128, CHUNK_T * 8], I16, isOutput=False)
    qlocw = nc.declare_dram_parameter("qlocw", [NCH * 128, CHUNK_T * 8], I16, isOutput=False)
    biasP = nc.declare_dram_parameter("biasP", [NT, 128, H], BF16, isOutput=False)
    ohP = nc.declare_dram_parameter("ohP", [NT, 128, 128], BF16, isOutput=False)
    y_out = nc.declare_dram_parameter("y", [LSH, C], F32, isOutput=True)

    with ExitStack() as ctx:
        tc = ctx.enter_context(tile.TileContext(nc))

        dram = ctx.enter_context(tc.tile_pool(name="dram", bufs=1, space="DRAM"))
        q_tab = dram.tile([LSH, C], BF16)
        kv_sh = dram.tile([LSH, 2 * C], BF16)
        # chunk-major full table: [NAG][NCORES][LSH4]
        kv_full = dram.tile([NCORES * LSH, 2 * C], BF16)

        # ---------------- constants + weights ----------------
        consts = ctx.enter_context(tc.tile_pool(name="consts", bufs=1))
        ident = consts.tile([128, 128], BF16, tag="ident")
        make_identity(nc, ident[:])
        ones_k1 = consts.tile([1, 128], BF16, tag="ones")
        nc.vector.memset(ones_k1[:], 1.0)
        eps_t = consts.tile([128, 1], F32, tag="eps")
        nc.vector.memset(eps_t[:], EPS)

        def bcast_load(param, tag):
            t = consts.tile([128, C], BF16, tag=tag)
            ap = param[:]
            src = bass.AP(tensor=ap.tensor, offset=ap.offset,
                          ap=[[0, 128], [1, C]])
            nc.sync.dma_start(out=t[:], in_=src)
            return t

        g1_bc, b1_bc = bcast_load(ln1g, "g1"), bcast_load(ln1b, "b1")
        g2_bc, b2_bc = bcast_load(ln2g, "g2"), bcast_load(ln2b, "b2")

        wts = ctx.enter_context(tc.tile_pool(name="wts", bufs=1))

        def wload(p, shape, tag):
            t = wts.tile(shape, BF16, tag=tag)
            nc.sync.dma_start(out=t[:], in_=p[:])
            return t

        wq_sb = wload(wq, [128, 4, C], "wq"); wk_sb = wload(wk, [128, 4, C], "wk")
        wv_sb = wload(wv, [128, 4, C], "wv"); wo_sb = wload(wo, [128, 4, C], "wo")
        w1_sb = wload(w1, [128, 4, HID], "w1"); w2_sb = wload(w2, [128, 8, C], "w2")
        bq_sb = wload(bqp, [1, C], "bq"); bk_sb = wload(bkp, [1, C], "bk")
        bv_sb = wload(bvp, [1, C], "bv"); bo_sb = wload(bop, [1, C], "bo")
        b1_sb = wload(b1p, [1, HID], "bb1"); b2_sb = wload(b2p, [1, C], "bb2")

        # ---------------- LN helper (fused tensor_scalar) ----------------
        def layernorm(pool, lnpool, xb, g_bc, bb_bc, tagp):
            stats = lnpool.tile([128, 6], F32, tag=tagp + "st")
            nc.vector.bn_stats(stats[:], xb[:])
            mv = lnpool.tile([128, 2], F32, tag=tagp + "mv")
            nc.vector.bn_aggr(mv[:], stats[:])
            sd = lnpool.tile([128, 1], F32, tag=tagp + "sd")
            nc.scalar.activation(sd[:], mv[:, 1:2], AF.Sqrt, bias=eps_t[:])
            rstd = lnpool.tile([128, 1], F32, tag=tagp + "rs")
            nc.vector.reciprocal(rstd[:], sd[:])
            z0 = pool.tile([128, C], BF16, tag=tagp + "z0")
            nc.vector.tensor_scalar(z0[:], xb[:], mv[:, 0:1], rstd[:],
                                    op0=ALU.subtract, op1=ALU.mult)
            z1 = pool.tile([128, C], BF16, tag=tagp + "z1")
            nc.vector.tensor_tensor(z1[:], z0[:], g_bc[:], op=ALU.mult)
            zb = pool.tile([128, C], BF16, tag=tagp + "zo")
            nc.vector.tensor_tensor(zb[:], z1[:], bb_bc[:], op=ALU.add)
            return zb

        # ---------------- phase B: LN1, zT, QKV (+chunked allgather) -------
        with ExitStack() as pctx:
            zT_pool = pctx.enter_context(tc.tile_pool(name="zT", bufs=1))
            zT = zT_pool.tile([128, 4, LSH], BF16)
            xp = pctx.enter_context(tc.tile_pool(name="xp", bufs=3))
            lnp = pctx.enter_context(tc.tile_pool(name="lnp", bufs=4))
            trp = pctx.enter_context(tc.tile_pool(name="trp", bufs=2, space="PSUM"))
            qkvp = pctx.enter_context(tc.tile_pool(name="qkvp", bufs=2, space="PSUM"))
            obp = pctx.enter_context(tc.tile_pool(name="obp", bufs=3))

            for ib in range(NBLK):
                sl = slice(ib * 128, (ib + 1) * 128)
                xb = xp.tile([128, C], F32, tag="xin")
                nc.sync.dma_start(out=xb[:], in_=x_c[sl, :])
                zb = layernorm(xp, lnp, xb, g1_bc, b1_bc, "l1")
                for g in range(4):
                    pt = trp.tile([128, 128], BF16)
                    nc.tensor.transpose(pt[:], zb[:, g * 128:(g + 1) * 128], ident[:])
                    nc.scalar.copy(zT[:, g, sl], pt[:])
                for w_sb, bias_sb, dst in (
                    (wq_sb, bq_sb, None),
                    (wk_sb, bk_sb, 0),
                    (wv_sb, bv_sb, 1),
                ):
                    ps = qkvp.tile([128, C], F32)
                    for g in range(4):
                        nc.tensor.matmul(ps[:], lhsT=zT[:, g, sl], rhs=w_sb[:, g, :],
                                         start=(g == 0), stop=False)
                    nc.tensor.matmul(ps[:], lhsT=ones_k1[:], rhs=bias_sb[:],
                                     start=False, stop=True)
                    ob = obp.tile([128, C], BF16)
                    nc.scalar.copy(ob[:], ps[:])
                    if dst is None:
                        nc.sync.dma_start(out=q_tab[sl, :], in_=ob[:])
                    else:
                        nc.sync.dma_start(out=kv_sh[sl, dst * C:(dst + 1) * C], in_=ob[:])
                # fire allgather for each finished quarter
                if (ib + 1) % BPA == 0:
                    j = (ib + 1) // BPA - 1
                    nc.gpsimd.collective_compute(
                        "AllGather", ALU.bypass,
                        replica_groups=[list(range(NCORES))],
                        ins=[kv_sh[j * LSH4:(j + 1) * LSH4, :]],
                        outs=[kv_full[j * NCORES * LSH4:(j + 1) * NCORES * LSH4, :]],
                    )

        # ---------------- phase E: edges + fused per-block tail ----------
        with ExitStack() as pctx:
            kvp = pctx.enter_context(tc.tile_pool(name="kvp", bufs=2))
            qgp = pctx.enter_context(tc.tile_pool(name="qgp", bufs=2))
            idxp = pctx.enter_context(tc.tile_pool(name="idxp", bufs=3))
            bp = pctx.enter_context(tc.tile_pool(name="bp", bufs=2))
            ohp_ = pctx.enter_context(tc.tile_pool(name="ohp", bufs=2))
            work = pctx.enter_context(tc.tile_pool(name="work", bufs=2))
            pop_ = pctx.enter_context(tc.tile_pool(name="pout", bufs=2, space="PSUM"))
            psp = pctx.enter_context(tc.tile_pool(name="pssum", bufs=2, space="PSUM"))
            trp2 = pctx.enter_context(tc.tile_pool(name="trp2", bufs=1, space="PSUM"))
            hp = pctx.enter_context(tc.tile_pool(name="hpsum", bufs=1, space="PSUM"))
            bop_ = pctx.enter_context(tc.tile_pool(name="bout", bufs=1, space="PSUM"))
            finp = pctx.enter_context(tc.tile_pool(name="finp", bufs=2))
            lnp2 = pctx.enter_context(tc.tile_pool(name="lnp2", bufs=4))

            kvb = qgb = bia = ohc = prod = sc = sc2 = pexp = wtc = p8c = None
            pout = pssum = None
            for t in range(NT):
                ch, slot = divmod(t, CHUNK_T)
                if slot == 0:
                    tiles_c = min(CHUNK_T, NT - ch * CHUNK_T)
                    n_idx = tiles_c * 128
                    cidx = idxp.tile([128, CHUNK_T * 8], I16, tag="cidx")
                    nc.sync.dma_start(out=cidx[:], in_=colw[ch * 128:(ch + 1) * 128, :])
                    qidx = idxp.tile([128, CHUNK_T * 8], I16, tag="qidx")
                    nc.sync.dma_start(out=qidx[:], in_=qlocw[ch * 128:(ch + 1) * 128, :])
                    kvb = kvp.tile([128, CHUNK_T, 2 * C], BF16)
                    nc.gpsimd.dma_gather(
                        out_ap=kvb[:, :tiles_c, :], in_ap=kv_full[:],
                        idxs_ap=cidx[:, :n_idx // 16],
                        num_idxs=n_idx, num_idxs_reg=n_idx, elem_size=2 * C,
                        single_packet=False)
                    qgb = qgp.tile([128, CHUNK_T, C], BF16)
                    nc.gpsimd.dma_gather(
                        out_ap=qgb[:, :tiles_c, :], in_ap=q_tab[:],
                        idxs_ap=qidx[:, :n_idx // 16],
                        num_idxs=n_idx, num_idxs_reg=n_idx, elem_size=C,
                        single_packet=False)
                    bia = bp.tile([128, CHUNK_T, H], BF16, tag="bia")
                    nc.sync.dma_start(
                        out=bia[:, :tiles_c, :],
                        in_=biasP[ch * CHUNK_T:ch * CHUNK_T + tiles_c, :, :]
                        .rearrange("t p h -> p t h"))
                    ohc = ohp_.tile([128, CHUNK_T, 128], BF16, tag="oh")
                    nc.sync.dma_start(
                        out=ohc[:, :tiles_c, :],
                        in_=ohP[ch * CHUNK_T:ch * CHUNK_T + tiles_c, :, :]
                        .rearrange("t p r -> p t r"))
                    # chunk-batched DVE/Act: prod, reduce, +bias, exp8, expand, wt
                    prod = work.tile([128, CHUNK_T, C], BF16, tag="prod")
                    nc.vector.tensor_tensor(prod[:, :tiles_c, :], kvb[:, :tiles_c, 0:C],
                                            qgb[:, :tiles_c, :], op=ALU.mult)
                    sc = work.tile([128, CHUNK_T, H], F32, tag="sc")
                    nc.vector.tensor_reduce(
                        sc[:, :tiles_c, :],
                        prod[:, :tiles_c, :].rearrange("p t (h d) -> p t h d", h=H),
                        axis=AX.X, op=ALU.add)
                    sc2 = work.tile([128, CHUNK_T, H], F32, tag="sc2")
                    nc.vector.tensor_tensor(sc2[:, :tiles_c, :], sc[:, :tiles_c, :],
                                            bia[:, :tiles_c, :], op=ALU.add)
                    p8c = work.tile([128, CHUNK_T, H], BF16, tag="p8")
                    nc.scalar.activation(p8c[:, :tiles_c, :], sc2[:, :tiles_c, :], AF.Exp)
                    pexp = work.tile([128, CHUNK_T, C], BF16, tag="pexp")
                    s2 = sc2[:, :tiles_c, :]
                    src_b = bass.AP(tensor=s2.tensor, offset=s2.offset,
                                    ap=[s2.ap[0], s2.ap[1], s2.ap[2], [0, D]])
                    nc.scalar.activation(
                        pexp[:, :tiles_c, :].rearrange("p t (h d) -> p t h d", h=H),
                        src_b, AF.Exp)
                    wtc = work.tile([128, CHUNK_T, C], BF16, tag="wt")
                    nc.vector.tensor_tensor(wtc[:, :tiles_c, :], kvb[:, :tiles_c, C:2 * C],
                                            pexp[:, :tiles_c, :], op=ALU.mult)

                rb, tb = divmod(t, T_BLK)
                if tb == 0:
                    pout = pop_.tile([128, C], F32)
                    pssum = psp.tile([128, H], F32)

                nc.tensor.matmul(pout[:], lhsT=ohc[:, slot, :], rhs=wtc[:, slot, :],
                                 start=(tb == 0), stop=(tb == T_BLK - 1))
                nc.tensor.matmul(pssum[:], lhsT=ohc[:, slot, :], rhs=p8c[:, slot, :],
                                 start=(tb == 0), stop=(tb == T_BLK - 1))

                if tb == T_BLK - 1:
                    # ---- fused block tail: att, Wo, residual, LN2, MLP ----
                    sl = slice(rb * 128, (rb + 1) * 128)
                    sm = finp.tile([128, H], F32, tag="sm")
                    nc.vector.tensor_scalar(sm[:], pssum[:], 1e-30, None, op0=ALU.max)
                    rec = finp.tile([128, H], F32, tag="rec")
                    nc.vector.reciprocal(rec[:], sm[:])
                    rexp = finp.tile([128, C], BF16, tag="rexp")
                    rap = bass.AP(tensor=rec.tensor, offset=rec[:].offset,
                                  ap=[rec[:].ap[0], [1, H], [0, D]])
                    nc.scalar.activation(
                        rexp[:].rearrange("p (h d) -> p h d", h=H), rap, AF.Copy)
                    att = finp.tile([128, C], BF16, tag="att")
                    nc.vector.tensor_tensor(att[:], pout[:], rexp[:], op=ALU.mult)
                    attT = finp.tile([128, 4, 128], BF16, tag="attT")
                    for g in range(4):
                        pt = trp2.tile([128, 128], BF16)
                        nc.tensor.transpose(pt[:], att[:, g * 128:(g + 1) * 128], ident[:])
                        nc.scalar.copy(attT[:, g, :], pt[:])
                    po = bop_.tile([128, C], F32, tag="po")
                    for g in range(4):
                        nc.tensor.matmul(po[:], lhsT=attT[:, g, :], rhs=wo_sb[:, g, :],
                                         start=(g == 0), stop=False)
                    nc.tensor.matmul(po[:], lhsT=ones_k1[:], rhs=bo_sb[:],
                                     start=False, stop=True)
                    xb2 = finp.tile([128, C], F32, tag="xb2")
                    nc.sync.dma_start(out=xb2[:], in_=x_c[sl, :])
                    x1t = finp.tile([128, C], F32, tag="x1t")
                    nc.vector.tensor_tensor(x1t[:], po[:], xb2[:], op=ALU.add)
                    # LN2 + MLP
                    z2 = layernorm(finp, lnp2, x1t, g2_bc, b2_bc, "l2")
                    z2T = finp.tile([128, 4, 128], BF16, tag="z2T")
                    for g in range(4):
                        pt = trp2.tile([128, 128], BF16)
                        nc.tensor.transpose(pt[:], z2[:, g * 128:(g + 1) * 128], ident[:])
                        nc.scalar.copy(z2T[:, g, :], pt[:])
                    ph = hp.tile([128, 8, 128], F32)
                    for chc in range(8):
                        csl = slice(chc * 128, (chc + 1) * 128)
                        for g in range(4):
                            nc.tensor.matmul(ph[:, chc, :], lhsT=w1_sb[:, g, csl],
                                             rhs=z2T[:, g, :], start=(g == 0), stop=False)
                        nc.tensor.matmul(ph[:, chc, :], lhsT=b1_sb[:, csl],
                                         rhs=ones_k1[:], start=False, stop=True)
                    hs = finp.tile([128, 8, 128], BF16, tag="hs")
                    nc.scalar.activation(hs[:], ph[:], AF.Silu)
                    py = bop_.tile([128, C], F32, tag="po")
                    for chc in range(8):
                        nc.tensor.matmul(py[:], lhsT=hs[:, chc, :], rhs=w2_sb[:, chc, :],
                                         start=(chc == 0), stop=False)
                    nc.tensor.matmul(py[:], lhsT=ones_k1[:], rhs=b2_sb[:],
                                     start=False, stop=True)
                    yt = finp.tile([128, C], F32, tag="yt")
                    nc.vector.tensor_tensor(yt[:], py[:], x1t[:], op=ALU.add)
                    nc.sync.dma_start(out=y_out[sl, :], in_=yt[:])

    nc.finalize()
    _split_multi_waits(nc)
    return nc


# --------------------------------------------------------------------------
# entry point
# --------------------------------------------------------------------------

def kernel(**inputs) -> np.ndarray:
    x = np.asarray(inputs["x"], np.float32)
    row = np.asarray(inputs["row_index"]).astype(np.int64)
    col = np.asarray(inputs["col_index"]).astype(np.int64)
    att_bias = np.asarray(inputs["att_bias"], np.float32)
    L = x.shape[0]
    LSH = L // NCORES

    T_BLK, NT, NCH, cores = _preprocess_edges(L, row, col, att_bias)
    w = _prep_weights(inputs)

    key = (L, T_BLK, NT, NCH)
    if key not in _prog_cache:
        _prog_cache[key] = _build_program(L, T_BLK, NT, NCH)
    nc = _prog_cache[key]

    in_maps = []
    for c in range(NCORES):
        m = dict(w)
        m["x_c"] = np.ascontiguousarray(x[c * LSH:(c + 1) * LSH])
        m.update(cores[c])
        in_maps.append(m)

    global LAST_EXEC_NS, LAST_RESULTS
    res = run_bass_kernel_spmd(nc, in_maps, list(range(NCORES)), trace=TRACE)
    LAST_RESULTS = res
    LAST_EXEC_NS = res.exec_time_ns
    return np.concatenate([res.results[c]["y"] for c in range(NCORES)], axis=0)
